# revision 26
# baseline (speedup 1.0000x reference)
"""Multi-head attention (B=4, S=1024, D=1024, H=16) on 8 TRN2 NeuronCores.

Sharding: batch (4) x head-half (2) -> 8 cores, zero cross-core traffic.
Core c handles batch b = c // 2 and heads [hh*8, hh*8+8) where hh = c % 2.
Each core computes a partial output y_part[s, e] (its 512 channels fed
through its slice of Wo); the host sums the two partials per batch and
adds the bias terms.

On-device pipeline per core (matmul operands bf16, accumulation fp32):
  V' = xv @ Wv'          [s, 512] natural layout + per-head ones column
  per head-pair j (pairs = dout chunks of 128 = 2 heads):
    QT_j = Wq_j' @ xq    [128 dout, 1024 s]   (weights pre-scaled 1/sqrt(dk))
    KT_j = Wk_j' @ xk    [128 dout, 1024 s]
    per q-chunk (2x512), per k-tile (8x128), both heads row-tiled in the PE:
      ST   = KhT.T @ QhT            [k 128, q 512]   (K=64, tile_position)
      E    = exp(ST + mask_bias)    (ACT, fused mask)
      psO += Vaug.T @ E             [65, q 512]  rows 0-63 = out_h^T, row 64 = denom
    concatT = psO[0:64] * (1/denom) (ACT exp(-ln) recip + gpsimd broadcast)
  y = concatT.T @ Wo'    [1024 s, 1024 e]
"""

import os
import sys

sys.path.insert(0, "/opt/trn_rl_repo")

import numpy as np
import ml_dtypes

BF16 = ml_dtypes.bfloat16

B, S, D = 4, 1024, 1024
HEADS = 16
DK = 64
P = 128
NCORES = 8
DCH = D // P       # 8 contraction chunks
PAIRS = 4          # head-pairs per core (8 heads / 2)
QN = 2             # q 512-chunks
KT = 8             # k tiles of 128
VW = 65            # V channels per head + ones column

_STATE = {}


def _build():
    """Build + compile the per-core Bass program (cached)."""
    if "nc" in _STATE:
        return _STATE["nc"]

    import concourse.bass as bass  # noqa: F401
    import concourse.mybir as mybir
    from concourse import bacc
    from concourse import tile

    f32 = mybir.dt.float32
    bf16 = mybir.dt.bfloat16
    AF = mybir.ActivationFunctionType
    ALU = mybir.AluOpType

    # Pin Exp/Ln to the one activation table containing both, so the
    # table-load pass never alternates tables between the softmax exp and
    # the ln/exp reciprocal (each ACT_TABLE_LOAD costs ~1.3us). Only the
    # chooser's view is filtered; table ids keep act_info.json order.
    _orig_tables = bacc.get_activation_tables

    def _pinned_tables(arch):
        t = dict(_orig_tables(arch))
        target = "natural_log_exp_and_others"
        if target in t:
            for k in t:
                if k != target:
                    t[k] = t[k] - {AF.Exp, AF.Ln}
        return t

    bacc.get_activation_tables = _pinned_tables

    nc = bacc.Bacc("TRN2", target_bir_lowering=False, debug=False)

    xq_d = nc.dram_tensor("xq", [D, S], bf16, kind="ExternalInput")
    xk_d = nc.dram_tensor("xk", [D, S], bf16, kind="ExternalInput")
    xv_d = nc.dram_tensor("xv", [D, S], bf16, kind="ExternalInput")
    wq_d = nc.dram_tensor("wq", [PAIRS, D, P], bf16, kind="ExternalInput")
    wk_d = nc.dram_tensor("wk", [PAIRS, D, P], bf16, kind="ExternalInput")
    wv_d = nc.dram_tensor("wv", [D, 512], bf16, kind="ExternalInput")
    wo_d = nc.dram_tensor("wo", [512, D], bf16, kind="ExternalInput")
    bq_d = nc.dram_tensor("bq", [P, PAIRS], f32, kind="ExternalInput")
    bk_d = nc.dram_tensor("bk", [P, PAIRS], f32, kind="ExternalInput")
    mb_d = nc.dram_tensor("mb", [P, KT], f32, kind="ExternalInput")
    y_d = nc.dram_tensor("y", [S, D], f32, kind="ExternalOutput")
    scr_d = nc.dram_tensor("scr", [P, 512], f32)  # warmup flush (Internal)

    from contextlib import ExitStack

    with tile.TileContext(nc) as tc, ExitStack() as ctx:
        const = ctx.enter_context(tc.tile_pool(name="const", bufs=1))
        # Resident tensors
        wv_sb = const.tile([P, DCH, 512], bf16)
        wo_sb = const.tile([P, PAIRS, D], bf16)
        xq_sb = const.tile([P, DCH, S], bf16)
        xk_sb = const.tile([P, DCH, S], bf16)
        xv_sb = const.tile([P, DCH, S], bf16)
        v_sb = const.tile([P, KT, 8 * VW], bf16)
        cat_sb = const.tile([P, PAIRS, S], bf16)
        bq_sb = const.tile([P, PAIRS], f32)
        bk_sb = const.tile([P, PAIRS], f32)
        mb_sb = const.tile([P, KT], f32)

        # Pools
        wqp = ctx.enter_context(tc.tile_pool(name="wqp", bufs=3))
        wkp = ctx.enter_context(tc.tile_pool(name="wkp", bufs=3))
        qtp = ctx.enter_context(tc.tile_pool(name="qtp", bufs=3))
        ktp = ctx.enter_context(tc.tile_pool(name="ktp", bufs=3))
        epool = ctx.enter_context(tc.tile_pool(name="epool", bufs=12))
        rpool = ctx.enter_context(tc.tile_pool(name="rpool", bufs=4))
        r2pool = ctx.enter_context(tc.tile_pool(name="r2pool", bufs=4))
        ypool = ctx.enter_context(tc.tile_pool(name="ypool", bufs=3))
        spool = ctx.enter_context(tc.tile_pool(name="spool", bufs=6))
        psacc = ctx.enter_context(tc.tile_pool(name="psacc", bufs=3, space="PSUM"))
        pssp = ctx.enter_context(tc.tile_pool(name="pssp", bufs=2, space="PSUM"))
        psop = ctx.enter_context(tc.tile_pool(name="psop", bufs=1, space="PSUM"))

        # --- PE warm-up: ~5us of dummy matmuls during the DMA ramp so HAM
        # reaches 8/8 before the real work; flushed to an internal DRAM
        # scratch so DCE keeps it. No input dependencies.
        wup = const.tile([P, P], bf16)
        nc.vector.memset(wup[:], 0.0)
        wup2 = const.tile([P, 512], bf16)
        nc.vector.memset(wup2[:], 0.0)
        psw = psacc.tile([P, 512], f32, tag="acc", name="psw")
        for i in range(24):
            nc.tensor.matmul(psw[:], wup[:], wup2[:],
                             start=(i == 0), stop=(i == 23))
        wflush = const.tile([P, 512], f32)
        nc.vector.tensor_copy(wflush[:], psw[:])
        nc.sync.dma_start(scr_d.ap(), wflush[:])

        # --- Phase V: V' projection (natural layout + ones columns) ---
        wv_r = wv_d.ap().rearrange("(d p) m -> d p m", p=P)
        xv_r = xv_d.ap().rearrange("(d p) s -> d p s", p=P)
        for d in range(DCH):
            nc.sync.dma_start(wv_sb[:, d], wv_r[d])
            nc.gpsimd.dma_start(xv_sb[:, d], xv_r[d])
        # memset can't emit a bf16-typed strided set here; stage ones in f32.
        ones_f32 = const.tile([P, KT, 8], f32)
        nc.vector.memset(ones_f32[:], 1.0)
        ones_view = v_sb.rearrange("p t (h c) -> p t h c", c=VW)[:, :, :, 64:65]
        nc.vector.tensor_copy(ones_view, ones_f32[:].unsqueeze(3))

        for st in range(KT):
            ps = psacc.tile([P, 512], f32, tag="acc", name=f"psv{st}")
            for d in range(DCH):
                nc.tensor.matmul(
                    ps[:],
                    xv_sb[:, d, st * P : (st + 1) * P],
                    wv_sb[:, d],
                    start=(d == 0),
                    stop=(d == DCH - 1),
                )
            vview = v_sb[:, st].rearrange("p (h c) -> p h c", c=VW)
            nc.vector.tensor_copy(
                vview[:, :, 0:64], ps[:].rearrange("p (h c) -> p h c", c=64)
            )

        # --- Resident loads for the pair phase ---
        xq_r = xq_d.ap().rearrange("(d p) s -> d p s", p=P)
        xk_r = xk_d.ap().rearrange("(d p) s -> d p s", p=P)
        nc.sync.dma_start(bq_sb[:], bq_d.ap())
        nc.sync.dma_start(bk_sb[:], bk_d.ap())
        nc.sync.dma_start(mb_sb[:], mb_d.ap())
        for d in range(DCH):
            nc.scalar.dma_start(xq_sb[:, d], xq_r[d])
        for d in range(DCH):
            nc.gpsimd.dma_start(xk_sb[:, d], xk_r[d])
        wq_r = wq_d.ap().rearrange("j (d p) m -> j p d m", p=P)
        wk_r = wk_d.ap().rearrange("j (d p) m -> j p d m", p=P)

        # --- Head-pair phase ---
        for j in range(PAIRS):
            wqt = wqp.tile([P, DCH, P], bf16, tag="wq", name=f"wq{j}")
            nc.sync.dma_start(wqt[:], wq_r[j])
            wkt = wkp.tile([P, DCH, P], bf16, tag="wk", name=f"wk{j}")
            nc.sync.dma_start(wkt[:], wk_r[j])

            qt = qtp.tile([P, S], bf16, tag="qt", name=f"qt{j}")
            ktt = ktp.tile([P, S], bf16, tag="kt", name=f"kt{j}")
            for n in range(QN):
                psq = psacc.tile([P, 512], f32, tag="acc", name=f"psq{j}_{n}")
                for d in range(DCH):
                    nc.tensor.matmul(
                        psq[:],
                        wqt[:, d],
                        xq_sb[:, d, n * 512 : (n + 1) * 512],
                        start=(d == 0),
                        stop=(d == DCH - 1),
                    )
                nc.vector.tensor_scalar_add(
                    qt[:, n * 512 : (n + 1) * 512], psq[:], bq_sb[:, j : j + 1]
                )
                psk = psacc.tile([P, 512], f32, tag="acc", name=f"psk{j}_{n}")
                for d in range(DCH):
                    nc.tensor.matmul(
                        psk[:],
                        wkt[:, d],
                        xk_sb[:, d, n * 512 : (n + 1) * 512],
                        start=(d == 0),
                        stop=(d == DCH - 1),
                    )
                nc.vector.tensor_scalar_add(
                    ktt[:, n * 512 : (n + 1) * 512], psk[:], bk_sb[:, j : j + 1]
                )

            # Attention: both heads of the pair interleaved so the K=64
            # score matmuls land on disjoint PE row groups back-to-back.
            # Their scores share one 2-bank PSUM tile (same k->partition
            # mapping, so one exp with one per-partition mask bias covers
            # both heads), halving ACT instruction count.
            for qn in range(QN):
                # All S/exp tiles first, then the two heads' AV accumulation
                # groups sequentially: only one psO slot is live at a time,
                # freeing a PSUM bank for a third QK accumulator.
                ets = []
                for kt in range(KT):
                    pss = pssp.tile(
                        [P, 2, 512], f32, tag="s", name=f"pss{j}_{qn}_{kt}"
                    )
                    for sub in range(2):
                        lo, hi = sub * 64, (sub + 1) * 64
                        nc.tensor.matmul(
                            pss[:, sub],
                            ktt[lo:hi, kt * P : (kt + 1) * P],
                            qt[lo:hi, qn * 512 : (qn + 1) * 512],
                            start=True,
                            stop=True,
                        )
                    et = epool.tile(
                        [P, 2, 512], bf16, tag="e", name=f"e{j}_{qn}_{kt}"
                    )
                    nc.scalar.activation(
                        et[:],
                        pss[:],
                        AF.Exp,
                        bias=mb_sb[:, kt : kt + 1],
                        scale=1.0,
                    )
                    ets.append(et)
                pso = [None, None]
                for sub in range(2):
                    h = j * 2 + sub
                    pso[sub] = psop.tile(
                        [VW, 512], f32, tag="o", name=f"pso{j}_{sub}_{qn}"
                    )
                    for kt in range(KT):
                        nc.tensor.matmul(
                            pso[sub][:],
                            v_sb[:, kt, h * VW : (h + 1) * VW],
                            ets[kt][:, sub],
                            start=(kt == 0),
                            stop=(kt == KT - 1),
                        )
                for sub in range(2):
                    lo, hi = sub * 64, (sub + 1) * 64
                    # Evict psO to SBUF right away (frees the PSUM bank in
                    # ~0.5us); the normalize chain then runs SBUF->SBUF off
                    # the critical path.
                    stg = spool.tile([VW, 512], f32, tag="stg", name=f"stg{j}_{sub}_{qn}")
                    nc.vector.tensor_copy(stg[:], pso[sub][:])
                    # 1/denom as exp(-ln(denom)) on ACT: ~5x cheaper than the
                    # iterative DVE reciprocal and keeps the chain off DVE.
                    lrow = rpool.tile([1, 512], f32, tag="l", name=f"l{j}_{sub}_{qn}")
                    nc.scalar.activation(lrow[:], pso[sub][64:65, :], AF.Ln)
                    rrow = rpool.tile([1, 512], f32, tag="r", name=f"r{j}_{sub}_{qn}")
                    nc.scalar.activation(rrow[:], lrow[:], AF.Exp, scale=-1.0)
                    r2 = r2pool.tile([64, 512], f32, tag="r2", name=f"r2{j}_{sub}_{qn}")
                    nc.gpsimd.partition_broadcast(r2[:], rrow[:])
                    nc.vector.tensor_tensor(
                        cat_sb[lo:hi, j, qn * 512 : (qn + 1) * 512],
                        stg[0:64, :],
                        r2[:],
                        op=ALU.mult,
                    )

        # --- Output projection ---
        # wo is only needed here; issuing it after the pair-weight DMAs keeps
        # the sync queue clear for pair 0 (the 1MB transfer was delaying wq0).
        nc.sync.dma_start(wo_sb[:], wo_d.ap().rearrange("(c p) e -> p c e", p=P))
        y_r = y_d.ap().rearrange("(st p) e -> st p e", p=P)
        for st in range(KT):
            for en in range(2):
                psy = psacc.tile([P, 512], f32, tag="acc", name=f"psy{st}_{en}")
                for cc in range(PAIRS):
                    nc.tensor.matmul(
                        psy[:],
                        cat_sb[:, cc, st * P : (st + 1) * P],
                        wo_sb[:, cc, en * 512 : (en + 1) * 512],
                        start=(cc == 0),
                        stop=(cc == PAIRS - 1),
                    )
                ysb = ypool.tile([P, 512], f32, tag="y", name=f"y{st}_{en}")
                nc.vector.tensor_copy(ysb[:], psy[:])
                nc.scalar.dma_start(y_r[st][:, en * 512 : (en + 1) * 512], ysb[:])

    nc.compile()
    _STATE["nc"] = nc
    return nc


def _shard(q, k, v, mask, Wq, bq, Wk, bk, Wv, bv, Wo, bo):
    """Build the 8 per-core input maps (host-side layout preparation)."""
    scale = 1.0 / np.sqrt(DK)
    in_maps = []
    for c in range(NCORES):
        b = c // 2
        hh = c % 2
        c0 = hh * 512
        wq_s = (Wq[c0 : c0 + 512, :] * scale).T  # [D, 512]
        wk_s = Wk[c0 : c0 + 512, :].T
        wv_s = Wv[c0 : c0 + 512, :].T
        wo_s = Wo[:, c0 : c0 + 512].T  # [512, D]
        mrow = mask[b, 0, 0, :]
        in_maps.append(
            {
                "xq": np.ascontiguousarray(q[b].T).astype(BF16),
                "xk": np.ascontiguousarray(k[b].T).astype(BF16),
                "xv": np.ascontiguousarray(v[b].T).astype(BF16),
                "wq": np.ascontiguousarray(
                    wq_s.reshape(D, PAIRS, P).transpose(1, 0, 2)
                ).astype(BF16),
                "wk": np.ascontiguousarray(
                    wk_s.reshape(D, PAIRS, P).transpose(1, 0, 2)
                ).astype(BF16),
                "wv": np.ascontiguousarray(wv_s).astype(BF16),
                "wo": np.ascontiguousarray(wo_s).astype(BF16),
                "bq": np.ascontiguousarray(
                    (bq[c0 : c0 + 512] * scale).reshape(PAIRS, P).T, dtype=np.float32
                ),
                "bk": np.ascontiguousarray(
                    bk[c0 : c0 + 512].reshape(PAIRS, P).T, dtype=np.float32
                ),
                "mb": np.ascontiguousarray(
                    np.where(mrow == 0, np.float32(-1e9), np.float32(0.0))
                    .astype(np.float32)
                    .reshape(KT, P)
                    .T
                ),
            }
        )
    return in_maps


def _gather(results, Wv, bv, Wo, bo):
    """Sum per-core partials into the full [B, S, D] output."""
    # Channel-bias correction folded out of the device kernel: the V bias
    # passes through softmax-weighted sums with total weight 1, so its
    # contribution to y is the constant row Wo @ bv.
    corr = (Wo.astype(np.float64) @ bv.astype(np.float64)).astype(np.float32)
    y = np.empty((B, S, D), dtype=np.float32)
    for b in range(B):
        y[b] = results[2 * b]["y"] + results[2 * b + 1]["y"] + corr + bo
    return y


def _run(trace=False, **inputs):
    import time

    from concourse.bass_utils import run_bass_kernel_spmd

    nc = _build()
    args = {k: np.asarray(v) for k, v in inputs.items()}
    in_maps = _shard(**args)
    last_err = None
    for attempt in range(3):
        try:
            res = run_bass_kernel_spmd(
                nc, in_maps, core_ids=list(range(NCORES)), trace=trace
            )
            break
        except Exception as e:  # device occasionally wedges; retry recovers
            last_err = e
            time.sleep(10 * (attempt + 1))
    else:
        raise last_err
    y = _gather(res.results, args["Wv"], args["bv"], args["Wo"], args["bo"])
    return y, res


def kernel(**inputs):
    y, _ = _run(trace=False, **inputs)
    return y


# revision 27
# speedup vs baseline: 1.0071x; 1.0071x over previous
"""Multi-head attention (B=4, S=1024, D=1024, H=16) on 8 TRN2 NeuronCores.

Sharding: batch (4) x head-half (2) -> 8 cores, zero cross-core traffic.
Core c handles batch b = c // 2 and heads [hh*8, hh*8+8) where hh = c % 2.
Each core computes a partial output y_part[s, e] (its 512 channels fed
through its slice of Wo); the host sums the two partials per batch and
adds the bias terms.

On-device pipeline per core (matmul operands bf16, accumulation fp32):
  V' = xv @ Wv'          [s, 512] natural layout + per-head ones column
  per head-pair j (pairs = dout chunks of 128 = 2 heads):
    QT_j = Wq_j' @ xq    [128 dout, 1024 s]   (weights pre-scaled 1/sqrt(dk))
    KT_j = Wk_j' @ xk    [128 dout, 1024 s]
    per q-chunk (2x512), per k-tile (8x128), both heads row-tiled in the PE:
      ST   = KhT.T @ QhT            [k 128, q 512]   (K=64, tile_position)
      E    = exp(ST + mask_bias)    (ACT, fused mask)
      psO += Vaug.T @ E             [65, q 512]  rows 0-63 = out_h^T, row 64 = denom
    concatT = psO[0:64] * (1/denom) (ACT exp(-ln) recip + gpsimd broadcast)
  y = concatT.T @ Wo'    [1024 s, 1024 e]
"""

import os
import sys

sys.path.insert(0, "/opt/trn_rl_repo")

import numpy as np
import ml_dtypes

BF16 = ml_dtypes.bfloat16

B, S, D = 4, 1024, 1024
HEADS = 16
DK = 64
P = 128
NCORES = 8
DCH = D // P       # 8 contraction chunks
PAIRS = 4          # head-pairs per core (8 heads / 2)
QN = 2             # q 512-chunks
KT = 8             # k tiles of 128
VW = 65            # V channels per head + ones column

_STATE = {}


def _build():
    """Build + compile the per-core Bass program (cached)."""
    if "nc" in _STATE:
        return _STATE["nc"]

    import concourse.bass as bass  # noqa: F401
    import concourse.mybir as mybir
    from concourse import bacc
    from concourse import tile

    f32 = mybir.dt.float32
    bf16 = mybir.dt.bfloat16
    AF = mybir.ActivationFunctionType
    ALU = mybir.AluOpType

    # Pin Exp/Ln to the one activation table containing both, so the
    # table-load pass never alternates tables between the softmax exp and
    # the ln/exp reciprocal (each ACT_TABLE_LOAD costs ~1.3us). Only the
    # chooser's view is filtered; table ids keep act_info.json order.
    _orig_tables = bacc.get_activation_tables

    def _pinned_tables(arch):
        t = dict(_orig_tables(arch))
        target = "natural_log_exp_and_others"
        if target in t:
            for k in t:
                if k != target:
                    t[k] = t[k] - {AF.Exp, AF.Ln}
        return t

    bacc.get_activation_tables = _pinned_tables

    nc = bacc.Bacc("TRN2", target_bir_lowering=False, debug=False)

    xq_d = nc.dram_tensor("xq", [D, S], bf16, kind="ExternalInput")
    xk_d = nc.dram_tensor("xk", [D, S], bf16, kind="ExternalInput")
    xv_d = nc.dram_tensor("xv", [D, S], bf16, kind="ExternalInput")
    wq_d = nc.dram_tensor("wq", [PAIRS, D, P], bf16, kind="ExternalInput")
    wk_d = nc.dram_tensor("wk", [PAIRS, D, P], bf16, kind="ExternalInput")
    wv_d = nc.dram_tensor("wv", [D, 512], bf16, kind="ExternalInput")
    wo_d = nc.dram_tensor("wo", [512, D], bf16, kind="ExternalInput")
    bq_d = nc.dram_tensor("bq", [P, PAIRS], f32, kind="ExternalInput")
    bk_d = nc.dram_tensor("bk", [P, PAIRS], f32, kind="ExternalInput")
    mb_d = nc.dram_tensor("mb", [P, KT], f32, kind="ExternalInput")
    y_d = nc.dram_tensor("y", [S, D], f32, kind="ExternalOutput")

    from contextlib import ExitStack

    with tile.TileContext(nc) as tc, ExitStack() as ctx:
        const = ctx.enter_context(tc.tile_pool(name="const", bufs=1))
        # Resident tensors
        wv_sb = const.tile([P, DCH, 512], bf16)
        wo_sb = const.tile([P, PAIRS, D], bf16)
        xq_sb = const.tile([P, DCH, S], bf16)
        xk_sb = const.tile([P, DCH, S], bf16)
        xv_sb = const.tile([P, DCH, S], bf16)
        v_sb = const.tile([P, KT, 8 * VW], bf16)
        cat_sb = const.tile([P, PAIRS, S], bf16)
        bq_sb = const.tile([P, PAIRS], f32)
        bk_sb = const.tile([P, PAIRS], f32)
        mb_sb = const.tile([P, KT], f32)

        # Pools
        wqp = ctx.enter_context(tc.tile_pool(name="wqp", bufs=3))
        wkp = ctx.enter_context(tc.tile_pool(name="wkp", bufs=3))
        qtp = ctx.enter_context(tc.tile_pool(name="qtp", bufs=3))
        ktp = ctx.enter_context(tc.tile_pool(name="ktp", bufs=3))
        epool = ctx.enter_context(tc.tile_pool(name="epool", bufs=12))
        rpool = ctx.enter_context(tc.tile_pool(name="rpool", bufs=4))
        r2pool = ctx.enter_context(tc.tile_pool(name="r2pool", bufs=4))
        ypool = ctx.enter_context(tc.tile_pool(name="ypool", bufs=3))
        spool = ctx.enter_context(tc.tile_pool(name="spool", bufs=6))
        psacc = ctx.enter_context(tc.tile_pool(name="psacc", bufs=3, space="PSUM"))
        pssp = ctx.enter_context(tc.tile_pool(name="pssp", bufs=2, space="PSUM"))
        psop = ctx.enter_context(tc.tile_pool(name="psop", bufs=1, space="PSUM"))

        # --- Phase V: V' projection (natural layout + ones columns) ---
        wv_r = wv_d.ap().rearrange("(d p) m -> d p m", p=P)
        xv_r = xv_d.ap().rearrange("(d p) s -> d p s", p=P)
        for d in range(DCH):
            nc.sync.dma_start(wv_sb[:, d], wv_r[d])
            nc.gpsimd.dma_start(xv_sb[:, d], xv_r[d])
        # memset can't emit a bf16-typed strided set here; stage ones in f32.
        ones_f32 = const.tile([P, KT, 8], f32)
        nc.vector.memset(ones_f32[:], 1.0)
        ones_view = v_sb.rearrange("p t (h c) -> p t h c", c=VW)[:, :, :, 64:65]
        nc.vector.tensor_copy(ones_view, ones_f32[:].unsqueeze(3))

        for st in range(KT):
            ps = psacc.tile([P, 512], f32, tag="acc", name=f"psv{st}")
            for d in range(DCH):
                nc.tensor.matmul(
                    ps[:],
                    xv_sb[:, d, st * P : (st + 1) * P],
                    wv_sb[:, d],
                    start=(d == 0),
                    stop=(d == DCH - 1),
                )
            vview = v_sb[:, st].rearrange("p (h c) -> p h c", c=VW)
            nc.vector.tensor_copy(
                vview[:, :, 0:64], ps[:].rearrange("p (h c) -> p h c", c=64)
            )

        # --- Resident loads for the pair phase ---
        xq_r = xq_d.ap().rearrange("(d p) s -> d p s", p=P)
        xk_r = xk_d.ap().rearrange("(d p) s -> d p s", p=P)
        nc.sync.dma_start(bq_sb[:], bq_d.ap())
        nc.sync.dma_start(bk_sb[:], bk_d.ap())
        nc.sync.dma_start(mb_sb[:], mb_d.ap())
        for d in range(DCH):
            nc.scalar.dma_start(xq_sb[:, d], xq_r[d])
        for d in range(DCH):
            nc.gpsimd.dma_start(xk_sb[:, d], xk_r[d])
        wq_r = wq_d.ap().rearrange("j (d p) m -> j p d m", p=P)
        wk_r = wk_d.ap().rearrange("j (d p) m -> j p d m", p=P)

        # --- Head-pair phase ---
        for j in range(PAIRS):
            wqt = wqp.tile([P, DCH, P], bf16, tag="wq", name=f"wq{j}")
            nc.sync.dma_start(wqt[:], wq_r[j])
            wkt = wkp.tile([P, DCH, P], bf16, tag="wk", name=f"wk{j}")
            nc.sync.dma_start(wkt[:], wk_r[j])

            qt = qtp.tile([P, S], bf16, tag="qt", name=f"qt{j}")
            ktt = ktp.tile([P, S], bf16, tag="kt", name=f"kt{j}")
            for n in range(QN):
                psq = psacc.tile([P, 512], f32, tag="acc", name=f"psq{j}_{n}")
                for d in range(DCH):
                    nc.tensor.matmul(
                        psq[:],
                        wqt[:, d],
                        xq_sb[:, d, n * 512 : (n + 1) * 512],
                        start=(d == 0),
                        stop=(d == DCH - 1),
                    )
                nc.vector.tensor_scalar_add(
                    qt[:, n * 512 : (n + 1) * 512], psq[:], bq_sb[:, j : j + 1]
                )
                psk = psacc.tile([P, 512], f32, tag="acc", name=f"psk{j}_{n}")
                for d in range(DCH):
                    nc.tensor.matmul(
                        psk[:],
                        wkt[:, d],
                        xk_sb[:, d, n * 512 : (n + 1) * 512],
                        start=(d == 0),
                        stop=(d == DCH - 1),
                    )
                nc.vector.tensor_scalar_add(
                    ktt[:, n * 512 : (n + 1) * 512], psk[:], bk_sb[:, j : j + 1]
                )

            # Attention: both heads of the pair interleaved so the K=64
            # score matmuls land on disjoint PE row groups back-to-back.
            # Their scores share one 2-bank PSUM tile (same k->partition
            # mapping, so one exp with one per-partition mask bias covers
            # both heads), halving ACT instruction count.
            for qn in range(QN):
                # All S/exp tiles first, then the two heads' AV accumulation
                # groups sequentially: only one psO slot is live at a time,
                # freeing a PSUM bank for a third QK accumulator.
                ets = []
                for kt in range(KT):
                    pss = pssp.tile(
                        [P, 2, 512], f32, tag="s", name=f"pss{j}_{qn}_{kt}"
                    )
                    for sub in range(2):
                        lo, hi = sub * 64, (sub + 1) * 64
                        nc.tensor.matmul(
                            pss[:, sub],
                            ktt[lo:hi, kt * P : (kt + 1) * P],
                            qt[lo:hi, qn * 512 : (qn + 1) * 512],
                            start=True,
                            stop=True,
                        )
                    et = epool.tile(
                        [P, 2, 512], bf16, tag="e", name=f"e{j}_{qn}_{kt}"
                    )
                    nc.scalar.activation(
                        et[:],
                        pss[:],
                        AF.Exp,
                        bias=mb_sb[:, kt : kt + 1],
                        scale=1.0,
                    )
                    ets.append(et)
                pso = [None, None]
                for sub in range(2):
                    h = j * 2 + sub
                    pso[sub] = psop.tile(
                        [VW, 512], f32, tag="o", name=f"pso{j}_{sub}_{qn}"
                    )
                    for kt in range(KT):
                        nc.tensor.matmul(
                            pso[sub][:],
                            v_sb[:, kt, h * VW : (h + 1) * VW],
                            ets[kt][:, sub],
                            start=(kt == 0),
                            stop=(kt == KT - 1),
                        )
                for sub in range(2):
                    lo, hi = sub * 64, (sub + 1) * 64
                    # Evict psO to SBUF right away (frees the PSUM bank in
                    # ~0.5us); the normalize chain then runs SBUF->SBUF off
                    # the critical path.
                    stg = spool.tile([VW, 512], f32, tag="stg", name=f"stg{j}_{sub}_{qn}")
                    nc.vector.tensor_copy(stg[:], pso[sub][:])
                    # 1/denom as exp(-ln(denom)) on ACT: ~5x cheaper than the
                    # iterative DVE reciprocal and keeps the chain off DVE.
                    lrow = rpool.tile([1, 512], f32, tag="l", name=f"l{j}_{sub}_{qn}")
                    nc.scalar.activation(lrow[:], pso[sub][64:65, :], AF.Ln)
                    rrow = rpool.tile([1, 512], f32, tag="r", name=f"r{j}_{sub}_{qn}")
                    nc.scalar.activation(rrow[:], lrow[:], AF.Exp, scale=-1.0)
                    r2 = r2pool.tile([64, 512], f32, tag="r2", name=f"r2{j}_{sub}_{qn}")
                    nc.gpsimd.partition_broadcast(r2[:], rrow[:])
                    nc.vector.tensor_tensor(
                        cat_sb[lo:hi, j, qn * 512 : (qn + 1) * 512],
                        stg[0:64, :],
                        r2[:],
                        op=ALU.mult,
                    )

        # --- Output projection ---
        # wo is only needed here; issuing it after the pair-weight DMAs keeps
        # the sync queue clear for pair 0 (the 1MB transfer was delaying wq0).
        nc.sync.dma_start(wo_sb[:], wo_d.ap().rearrange("(c p) e -> p c e", p=P))
        y_r = y_d.ap().rearrange("(st p) e -> st p e", p=P)
        for st in range(KT):
            for en in range(2):
                psy = psacc.tile([P, 512], f32, tag="acc", name=f"psy{st}_{en}")
                for cc in range(PAIRS):
                    nc.tensor.matmul(
                        psy[:],
                        cat_sb[:, cc, st * P : (st + 1) * P],
                        wo_sb[:, cc, en * 512 : (en + 1) * 512],
                        start=(cc == 0),
                        stop=(cc == PAIRS - 1),
                    )
                ysb = ypool.tile([P, 512], f32, tag="y", name=f"y{st}_{en}")
                nc.vector.tensor_copy(ysb[:], psy[:])
                nc.scalar.dma_start(y_r[st][:, en * 512 : (en + 1) * 512], ysb[:])

    nc.compile()
    _STATE["nc"] = nc
    return nc


def _shard(q, k, v, mask, Wq, bq, Wk, bk, Wv, bv, Wo, bo):
    """Build the 8 per-core input maps (host-side layout preparation)."""
    scale = 1.0 / np.sqrt(DK)
    in_maps = []
    for c in range(NCORES):
        b = c // 2
        hh = c % 2
        c0 = hh * 512
        wq_s = (Wq[c0 : c0 + 512, :] * scale).T  # [D, 512]
        wk_s = Wk[c0 : c0 + 512, :].T
        wv_s = Wv[c0 : c0 + 512, :].T
        wo_s = Wo[:, c0 : c0 + 512].T  # [512, D]
        mrow = mask[b, 0, 0, :]
        in_maps.append(
            {
                "xq": np.ascontiguousarray(q[b].T).astype(BF16),
                "xk": np.ascontiguousarray(k[b].T).astype(BF16),
                "xv": np.ascontiguousarray(v[b].T).astype(BF16),
                "wq": np.ascontiguousarray(
                    wq_s.reshape(D, PAIRS, P).transpose(1, 0, 2)
                ).astype(BF16),
                "wk": np.ascontiguousarray(
                    wk_s.reshape(D, PAIRS, P).transpose(1, 0, 2)
                ).astype(BF16),
                "wv": np.ascontiguousarray(wv_s).astype(BF16),
                "wo": np.ascontiguousarray(wo_s).astype(BF16),
                "bq": np.ascontiguousarray(
                    (bq[c0 : c0 + 512] * scale).reshape(PAIRS, P).T, dtype=np.float32
                ),
                "bk": np.ascontiguousarray(
                    bk[c0 : c0 + 512].reshape(PAIRS, P).T, dtype=np.float32
                ),
                "mb": np.ascontiguousarray(
                    np.where(mrow == 0, np.float32(-1e9), np.float32(0.0))
                    .astype(np.float32)
                    .reshape(KT, P)
                    .T
                ),
            }
        )
    return in_maps


def _gather(results, Wv, bv, Wo, bo):
    """Sum per-core partials into the full [B, S, D] output."""
    # Channel-bias correction folded out of the device kernel: the V bias
    # passes through softmax-weighted sums with total weight 1, so its
    # contribution to y is the constant row Wo @ bv.
    corr = (Wo.astype(np.float64) @ bv.astype(np.float64)).astype(np.float32)
    y = np.empty((B, S, D), dtype=np.float32)
    for b in range(B):
        y[b] = results[2 * b]["y"] + results[2 * b + 1]["y"] + corr + bo
    return y


def _run(trace=False, **inputs):
    import time

    from concourse.bass_utils import run_bass_kernel_spmd

    nc = _build()
    args = {k: np.asarray(v) for k, v in inputs.items()}
    in_maps = _shard(**args)
    last_err = None
    for attempt in range(3):
        try:
            res = run_bass_kernel_spmd(
                nc, in_maps, core_ids=list(range(NCORES)), trace=trace
            )
            break
        except Exception as e:  # device occasionally wedges; retry recovers
            last_err = e
            time.sleep(10 * (attempt + 1))
    else:
        raise last_err
    y = _gather(res.results, args["Wv"], args["bv"], args["Wo"], args["bo"])
    return y, res


def kernel(**inputs):
    y, _ = _run(trace=False, **inputs)
    return y


# revision 28
# speedup vs baseline: 1.0365x; 1.0291x over previous
"""Multi-head attention (B=4, S=1024, D=1024, H=16) on 8 TRN2 NeuronCores.

Sharding: batch (4) x head-half (2) -> 8 cores, zero cross-core traffic.
Core c handles batch b = c // 2 and heads [hh*8, hh*8+8) where hh = c % 2.
Each core computes a partial output y_part[s, e] (its 512 channels fed
through its slice of Wo); the host sums the two partials per batch and
adds the bias terms.

On-device pipeline per core (matmul operands bf16, accumulation fp32):
  V' = xv @ Wv'          [s, 512] natural layout + per-head ones column
  per head-pair j (pairs = dout chunks of 128 = 2 heads):
    QT_j = Wq_j' @ xq    [128 dout, 1024 s]   (weights pre-scaled 1/sqrt(dk))
    KT_j = Wk_j' @ xk    [128 dout, 1024 s]
    per q-chunk (2x512), per k-tile (8x128), both heads row-tiled in the PE:
      ST   = KhT.T @ QhT            [k 128, q 512]   (K=64, tile_position)
      E    = exp(ST + mask_bias)    (ACT, fused mask)
      psO += Vaug.T @ E             [65, q 512]  rows 0-63 = out_h^T, row 64 = denom
    concatT = psO[0:64] * (1/denom) (ACT exp(-ln) recip + gpsimd broadcast)
  y = concatT.T @ Wo'    [1024 s, 1024 e]
"""

import os
import sys

sys.path.insert(0, "/opt/trn_rl_repo")

import numpy as np
import ml_dtypes

BF16 = ml_dtypes.bfloat16

B, S, D = 4, 1024, 1024
HEADS = 16
DK = 64
P = 128
NCORES = 8
DCH = D // P       # 8 contraction chunks
PAIRS = 4          # head-pairs per core (8 heads / 2)
QN = 2             # q 512-chunks
KT = 8             # k tiles of 128
VW = 65            # V channels per head + ones column

_STATE = {}


def _build():
    """Build + compile the per-core Bass program (cached)."""
    if "nc" in _STATE:
        return _STATE["nc"]

    import concourse.bass as bass  # noqa: F401
    import concourse.mybir as mybir
    from concourse import bacc
    from concourse import tile

    f32 = mybir.dt.float32
    bf16 = mybir.dt.bfloat16
    AF = mybir.ActivationFunctionType
    ALU = mybir.AluOpType

    # Pin Exp/Ln to the one activation table containing both, so the
    # table-load pass never alternates tables between the softmax exp and
    # the ln/exp reciprocal (each ACT_TABLE_LOAD costs ~1.3us). Only the
    # chooser's view is filtered; table ids keep act_info.json order.
    _orig_tables = bacc.get_activation_tables

    def _pinned_tables(arch):
        t = dict(_orig_tables(arch))
        target = "natural_log_exp_and_others"
        if target in t:
            for k in t:
                if k != target:
                    t[k] = t[k] - {AF.Exp, AF.Ln}
        return t

    bacc.get_activation_tables = _pinned_tables

    nc = bacc.Bacc("TRN2", target_bir_lowering=False, debug=False)

    xq_d = nc.dram_tensor("xq", [D, S], bf16, kind="ExternalInput")
    xk_d = nc.dram_tensor("xk", [D, S], bf16, kind="ExternalInput")
    xv_d = nc.dram_tensor("xv", [D, S], bf16, kind="ExternalInput")
    wq_d = nc.dram_tensor("wq", [PAIRS, D, P], bf16, kind="ExternalInput")
    wk_d = nc.dram_tensor("wk", [PAIRS, D, P], bf16, kind="ExternalInput")
    wv_d = nc.dram_tensor("wv", [D, 512], bf16, kind="ExternalInput")
    wo_d = nc.dram_tensor("wo", [512, D], bf16, kind="ExternalInput")
    bq_d = nc.dram_tensor("bq", [P, PAIRS], f32, kind="ExternalInput")
    bk_d = nc.dram_tensor("bk", [P, PAIRS], f32, kind="ExternalInput")
    mb_d = nc.dram_tensor("mb", [P, KT], f32, kind="ExternalInput")
    y_d = nc.dram_tensor("y", [S, D], f32, kind="ExternalOutput")

    from contextlib import ExitStack

    with tile.TileContext(nc) as tc, ExitStack() as ctx:
        const = ctx.enter_context(tc.tile_pool(name="const", bufs=1))
        # Resident tensors
        wv_sb = const.tile([P, DCH, 512], bf16)
        wo_sb = const.tile([P, PAIRS, D], bf16)
        xq_sb = const.tile([P, DCH, S], bf16)
        xk_sb = const.tile([P, DCH, S], bf16)
        xv_sb = const.tile([P, DCH, S], bf16)
        v_sb = const.tile([P, KT, 8 * VW], bf16)
        cat_sb = const.tile([P, PAIRS, S], bf16)
        bq_sb = const.tile([P, PAIRS], f32)
        bk_sb = const.tile([P, PAIRS], f32)
        mb_sb = const.tile([P, KT], f32)

        # Pools
        wqp = ctx.enter_context(tc.tile_pool(name="wqp", bufs=3))
        wkp = ctx.enter_context(tc.tile_pool(name="wkp", bufs=3))
        qtp = ctx.enter_context(tc.tile_pool(name="qtp", bufs=3))
        ktp = ctx.enter_context(tc.tile_pool(name="ktp", bufs=3))
        epool = ctx.enter_context(tc.tile_pool(name="epool", bufs=16))
        rpool = ctx.enter_context(tc.tile_pool(name="rpool", bufs=4))
        r2pool = ctx.enter_context(tc.tile_pool(name="r2pool", bufs=4))
        ypool = ctx.enter_context(tc.tile_pool(name="ypool", bufs=3))
        spool = ctx.enter_context(tc.tile_pool(name="spool", bufs=6))
        psacc = ctx.enter_context(tc.tile_pool(name="psacc", bufs=3, space="PSUM"))
        pssp = ctx.enter_context(tc.tile_pool(name="pssp", bufs=2, space="PSUM"))
        psop = ctx.enter_context(tc.tile_pool(name="psop", bufs=1, space="PSUM"))

        # --- Phase V: V' projection (natural layout + ones columns) ---
        wv_r = wv_d.ap().rearrange("(d p) m -> d p m", p=P)
        xv_r = xv_d.ap().rearrange("(d p) s -> d p s", p=P)
        for d in range(DCH):
            nc.sync.dma_start(wv_sb[:, d], wv_r[d])
            nc.gpsimd.dma_start(xv_sb[:, d], xv_r[d])
        # memset can't emit a bf16-typed strided set here; stage ones in f32.
        ones_f32 = const.tile([P, KT, 8], f32)
        nc.vector.memset(ones_f32[:], 1.0)
        ones_view = v_sb.rearrange("p t (h c) -> p t h c", c=VW)[:, :, :, 64:65]
        nc.vector.tensor_copy(ones_view, ones_f32[:].unsqueeze(3))

        for st in range(KT):
            ps = psacc.tile([P, 512], f32, tag="acc", name=f"psv{st}")
            for d in range(DCH):
                nc.tensor.matmul(
                    ps[:],
                    xv_sb[:, d, st * P : (st + 1) * P],
                    wv_sb[:, d],
                    start=(d == 0),
                    stop=(d == DCH - 1),
                )
            vview = v_sb[:, st].rearrange("p (h c) -> p h c", c=VW)
            nc.vector.tensor_copy(
                vview[:, :, 0:64], ps[:].rearrange("p (h c) -> p h c", c=64)
            )

        # --- Resident loads for the pair phase ---
        xq_r = xq_d.ap().rearrange("(d p) s -> d p s", p=P)
        xk_r = xk_d.ap().rearrange("(d p) s -> d p s", p=P)
        for d in range(DCH):
            nc.scalar.dma_start(xq_sb[:, d], xq_r[d])
        nc.scalar.dma_start(bq_sb[:], bq_d.ap())
        nc.scalar.dma_start(bk_sb[:], bk_d.ap())
        nc.scalar.dma_start(mb_sb[:], mb_d.ap())
        for d in range(DCH):
            nc.sync.dma_start(xk_sb[:, d], xk_r[d])
        wq_r = wq_d.ap().rearrange("j (d p) m -> j p d m", p=P)
        wk_r = wk_d.ap().rearrange("j (d p) m -> j p d m", p=P)

        # --- Head-pair phase ---
        for j in range(PAIRS):
            wqt = wqp.tile([P, DCH, P], bf16, tag="wq", name=f"wq{j}")
            nc.sync.dma_start(wqt[:], wq_r[j])
            wkt = wkp.tile([P, DCH, P], bf16, tag="wk", name=f"wk{j}")
            nc.sync.dma_start(wkt[:], wk_r[j])

            qt = qtp.tile([P, S], bf16, tag="qt", name=f"qt{j}")
            ktt = ktp.tile([P, S], bf16, tag="kt", name=f"kt{j}")
            for n in range(QN):
                psq = psacc.tile([P, 512], f32, tag="acc", name=f"psq{j}_{n}")
                for d in range(DCH):
                    nc.tensor.matmul(
                        psq[:],
                        wqt[:, d],
                        xq_sb[:, d, n * 512 : (n + 1) * 512],
                        start=(d == 0),
                        stop=(d == DCH - 1),
                    )
                nc.vector.tensor_scalar_add(
                    qt[:, n * 512 : (n + 1) * 512], psq[:], bq_sb[:, j : j + 1]
                )
                psk = psacc.tile([P, 512], f32, tag="acc", name=f"psk{j}_{n}")
                for d in range(DCH):
                    nc.tensor.matmul(
                        psk[:],
                        wkt[:, d],
                        xk_sb[:, d, n * 512 : (n + 1) * 512],
                        start=(d == 0),
                        stop=(d == DCH - 1),
                    )
                nc.vector.tensor_scalar_add(
                    ktt[:, n * 512 : (n + 1) * 512], psk[:], bk_sb[:, j : j + 1]
                )

            # Attention: both heads of the pair interleaved so the K=64
            # score matmuls land on disjoint PE row groups back-to-back.
            # Their scores share one 2-bank PSUM tile (same k->partition
            # mapping, so one exp with one per-partition mask bias covers
            # both heads), halving ACT instruction count.
            for qn in range(QN):
                # All S/exp tiles first, then the two heads' AV accumulation
                # groups sequentially: only one psO slot is live at a time,
                # freeing a PSUM bank for a third QK accumulator.
                ets = []
                for kt in range(KT):
                    pss = pssp.tile(
                        [P, 2, 512], f32, tag="s", name=f"pss{j}_{qn}_{kt}"
                    )
                    for sub in range(2):
                        lo, hi = sub * 64, (sub + 1) * 64
                        nc.tensor.matmul(
                            pss[:, sub],
                            ktt[lo:hi, kt * P : (kt + 1) * P],
                            qt[lo:hi, qn * 512 : (qn + 1) * 512],
                            start=True,
                            stop=True,
                        )
                    et = epool.tile(
                        [P, 2, 512], bf16, tag="e", name=f"e{j}_{qn}_{kt}"
                    )
                    nc.scalar.activation(
                        et[:],
                        pss[:],
                        AF.Exp,
                        bias=mb_sb[:, kt : kt + 1],
                        scale=1.0,
                    )
                    ets.append(et)
                pso = [None, None]
                for sub in range(2):
                    h = j * 2 + sub
                    pso[sub] = psop.tile(
                        [VW, 512], f32, tag="o", name=f"pso{j}_{sub}_{qn}"
                    )
                    for kt in range(KT):
                        nc.tensor.matmul(
                            pso[sub][:],
                            v_sb[:, kt, h * VW : (h + 1) * VW],
                            ets[kt][:, sub],
                            start=(kt == 0),
                            stop=(kt == KT - 1),
                        )
                for sub in range(2):
                    lo, hi = sub * 64, (sub + 1) * 64
                    # Evict psO to SBUF right away (frees the PSUM bank in
                    # ~0.5us); the normalize chain then runs SBUF->SBUF off
                    # the critical path.
                    stg = spool.tile([VW, 512], f32, tag="stg", name=f"stg{j}_{sub}_{qn}")
                    nc.vector.tensor_copy(stg[:], pso[sub][:])
                    # 1/denom as exp(-ln(denom)) on ACT: ~5x cheaper than the
                    # iterative DVE reciprocal and keeps the chain off DVE.
                    lrow = rpool.tile([1, 512], f32, tag="l", name=f"l{j}_{sub}_{qn}")
                    nc.scalar.activation(lrow[:], pso[sub][64:65, :], AF.Ln)
                    rrow = rpool.tile([1, 512], f32, tag="r", name=f"r{j}_{sub}_{qn}")
                    nc.scalar.activation(rrow[:], lrow[:], AF.Exp, scale=-1.0)
                    r2 = r2pool.tile([64, 512], f32, tag="r2", name=f"r2{j}_{sub}_{qn}")
                    nc.gpsimd.partition_broadcast(r2[:], rrow[:])
                    nc.vector.tensor_tensor(
                        cat_sb[lo:hi, j, qn * 512 : (qn + 1) * 512],
                        stg[0:64, :],
                        r2[:],
                        op=ALU.mult,
                    )

        # --- Output projection ---
        # wo is only needed here; issuing it after the pair-weight DMAs keeps
        # the sync queue clear for pair 0 (the 1MB transfer was delaying wq0).
        nc.sync.dma_start(wo_sb[:], wo_d.ap().rearrange("(c p) e -> p c e", p=P))
        y_r = y_d.ap().rearrange("(st p) e -> st p e", p=P)
        for st in range(KT):
            for en in range(2):
                psy = psacc.tile([P, 512], f32, tag="acc", name=f"psy{st}_{en}")
                for cc in range(PAIRS):
                    nc.tensor.matmul(
                        psy[:],
                        cat_sb[:, cc, st * P : (st + 1) * P],
                        wo_sb[:, cc, en * 512 : (en + 1) * 512],
                        start=(cc == 0),
                        stop=(cc == PAIRS - 1),
                    )
                ysb = ypool.tile([P, 512], f32, tag="y", name=f"y{st}_{en}")
                nc.vector.tensor_copy(ysb[:], psy[:])
                nc.scalar.dma_start(y_r[st][:, en * 512 : (en + 1) * 512], ysb[:])

    nc.compile()
    _STATE["nc"] = nc
    return nc


def _shard(q, k, v, mask, Wq, bq, Wk, bk, Wv, bv, Wo, bo):
    """Build the 8 per-core input maps (host-side layout preparation)."""
    scale = 1.0 / np.sqrt(DK)
    in_maps = []
    for c in range(NCORES):
        b = c // 2
        hh = c % 2
        c0 = hh * 512
        wq_s = (Wq[c0 : c0 + 512, :] * scale).T  # [D, 512]
        wk_s = Wk[c0 : c0 + 512, :].T
        wv_s = Wv[c0 : c0 + 512, :].T
        wo_s = Wo[:, c0 : c0 + 512].T  # [512, D]
        mrow = mask[b, 0, 0, :]
        in_maps.append(
            {
                "xq": np.ascontiguousarray(q[b].T).astype(BF16),
                "xk": np.ascontiguousarray(k[b].T).astype(BF16),
                "xv": np.ascontiguousarray(v[b].T).astype(BF16),
                "wq": np.ascontiguousarray(
                    wq_s.reshape(D, PAIRS, P).transpose(1, 0, 2)
                ).astype(BF16),
                "wk": np.ascontiguousarray(
                    wk_s.reshape(D, PAIRS, P).transpose(1, 0, 2)
                ).astype(BF16),
                "wv": np.ascontiguousarray(wv_s).astype(BF16),
                "wo": np.ascontiguousarray(wo_s).astype(BF16),
                "bq": np.ascontiguousarray(
                    (bq[c0 : c0 + 512] * scale).reshape(PAIRS, P).T, dtype=np.float32
                ),
                "bk": np.ascontiguousarray(
                    bk[c0 : c0 + 512].reshape(PAIRS, P).T, dtype=np.float32
                ),
                "mb": np.ascontiguousarray(
                    np.where(mrow == 0, np.float32(-1e9), np.float32(0.0))
                    .astype(np.float32)
                    .reshape(KT, P)
                    .T
                ),
            }
        )
    return in_maps


def _gather(results, Wv, bv, Wo, bo):
    """Sum per-core partials into the full [B, S, D] output."""
    # Channel-bias correction folded out of the device kernel: the V bias
    # passes through softmax-weighted sums with total weight 1, so its
    # contribution to y is the constant row Wo @ bv.
    corr = (Wo.astype(np.float64) @ bv.astype(np.float64)).astype(np.float32)
    y = np.empty((B, S, D), dtype=np.float32)
    for b in range(B):
        y[b] = results[2 * b]["y"] + results[2 * b + 1]["y"] + corr + bo
    return y


def _run(trace=False, **inputs):
    import time

    from concourse.bass_utils import run_bass_kernel_spmd

    nc = _build()
    args = {k: np.asarray(v) for k, v in inputs.items()}
    in_maps = _shard(**args)
    last_err = None
    for attempt in range(3):
        try:
            res = run_bass_kernel_spmd(
                nc, in_maps, core_ids=list(range(NCORES)), trace=trace
            )
            break
        except Exception as e:  # device occasionally wedges; retry recovers
            last_err = e
            time.sleep(10 * (attempt + 1))
    else:
        raise last_err
    y = _gather(res.results, args["Wv"], args["bv"], args["Wo"], args["bo"])
    return y, res


def kernel(**inputs):
    y, _ = _run(trace=False, **inputs)
    return y
